# revision 66
# baseline (speedup 1.0000x reference)
"""Trainium2 Bass kernel for nn_CpxRNN: 64-step RNN over B=4096 samples,
data-parallel across 8 NeuronCores (512 samples/core).

Math (per core, b = sample, columns of every on-chip tile):
  state kept transposed+shifted: nh = (elu(z)+1)^T, four independent
  quarter-chains of 128 samples (tiles [128, 256]; hidden chunk m in cols
  [128m, 128m+128), hidden unit i = 128*m + p).  The exp+stt elu tail of
  quarter q overlaps the other quarters' matmuls, keeping PE
  throughput-bound (~32 x 53ns matmuls/step) instead of latency-bound.
  elu(z)+1 == min(exp(z), max(z+1, 1))  -- 1 ACT + 1 DVE op, bias folded
  into the K=3 augmented input matmul (all "-1" shift corrections folded
  into host-precomputed biases b~ = b - colsum(W)).
  One-hot input term reduced to rank-1: prevoh @ W_in = W_in[0] + x*delta.
  Heads (logits 2 + phase 4 rows) for step t-2 are emitted during step t
  (2 matmuls per quarter at 32-aligned partition offsets, 4 steps per PSUM
  bank, bank evicted bf16 via split DVE/ACT copy + SBUF DMA shuffle).
  The final bank (steps 60..63) is post-processed directly from PSUM.
  Post phase: logp(t,b) = -softplus((1-2x)*D) with D = L1-L0 from a
  pattern matmul over elu'd head tiles (binary log-softmax identity
  removes the S / L0 / log-softmax tensors); softplus = Ln(Exp(u)+1) on
  one pre-loaded activation table (no mid-run table switch); final sums
  via one PE matmul + a GPSIMD partition-reduce straight into the DMA
  staging tile.  tau 0..2 head post-processing is injected into scan
  slack; the final PSUM bank (steps 60..63) is consumed in place.
  All constants ride in two packed dram tensors (5 input DMAs total);
  dummy warmup matmuls ramp the PE p-state during the DMA wait.
"""

import sys

sys.path.insert(0, "/opt/trn_rl_repo")

from contextlib import ExitStack

import ml_dtypes
import numpy as np

import concourse.bass as bass
import concourse.tile as tile
from concourse import bacc, mybir
from concourse.bass_utils import run_bass_kernel_spmd

L = 64
H = 256
B = 4096
NCORES = 8
BL = B // NCORES  # 512
NQ = 4            # quarter-chains per core, 128 samples each
F32 = mybir.dt.float32
BF16 = mybir.dt.bfloat16
AF = mybir.ActivationFunctionType
OP = mybir.AluOpType
BF = ml_dtypes.bfloat16

# packed-constant column layouts (bf16 tensor "cb", f32 tensor "cf");
# (name, ncols).  Rows beyond each entry's natural height are zero.
_CB_LAYOUT = [
    ("wc0", 256), ("wc1", 256), ("db", 256), ("nh0", 256),      # scan-crit
    ("neg1b", 1), ("whp0", 128), ("whp1", 128), ("wh0", 6), ("wh1", 6),
    ("ones128", 1),
    ("pattD", 256), ("pattW2", 512), ("pattD_L", 64), ("pattW2_L", 128),
]
_CB_CRIT = 1281   # cols holding scan-critical constants (first DMA)
_CF_LAYOUT = [
    ("neg1", 1), ("bias_head", 1), ("bias_head_p1", 1),
    ("bias_y", 1), ("bias_y_p1", 1),
    ("bias_head_L", 1), ("bias_head_p1_L", 1), ("sx", 512),
]


def _offsets(layout):
    offs, c = {}, 0
    for k, n in layout:
        offs[k] = (c, c + n)
        c += n
    return offs, c


_CB_OFF, _CB_COLS = _offsets(_CB_LAYOUT)
_CF_OFF, _CF_COLS = _offsets(_CF_LAYOUT)

# ---------------------------------------------------------------- host side


def _host_constants(W_in, W_carry, b_carry, W_prob, b_prob, W_ph1, b_ph1,
                    W_ph2, b_ph2):
    W_in = W_in.astype(np.float64)
    W_carry = W_carry.astype(np.float64)
    # state bias: b_carry - colsum(W_carry) + W_in[0]
    bias_state_vec = b_carry - W_carry.sum(0) + W_in[0]

    cb = {}
    # initial shifted state (t=0 uses zero prev input, zero h), quarter layout
    nh0_vec = np.where(b_carry > 0, b_carry, np.expm1(b_carry)) + 1.0  # [256]
    nh0 = np.empty((128, 256), np.float32)
    for m in range(2):
        nh0[:, 128 * m:128 * m + 128] = nh0_vec[128 * m:128 * m + 128][:, None]
    cb["nh0"] = nh0

    wcf = W_carry.astype(np.float32)
    cb["wc0"] = wcf[0:128]
    cb["wc1"] = wcf[128:256]
    # K=3 augmented input matmul: lhsT rows = [delta; bias_hi; bias_lo]
    # (bias split so the bf16 lhsT carries it at ~f32 accuracy)
    bp1 = bias_state_vec + 1.0   # PSUM carries z+1 for the fused elu tail
    b_hi = bp1.astype(np.float32).astype(BF).astype(np.float64)
    b_lo = bp1 - b_hi
    cb["db"] = np.stack([W_in[1] - W_in[0], b_hi, b_lo]).astype(np.float32)

    W_head = np.concatenate([W_prob, W_ph1], axis=1)          # [256, 6]
    b_head = np.concatenate([b_prob, b_ph1])                  # [6]
    cb["wh0"] = W_head[0:128].astype(np.float32)
    cb["wh1"] = W_head[128:256].astype(np.float32)
    whp = np.zeros((256, 128), np.float32)
    whp[:, :6] = W_head
    cb["whp0"] = whp[0:128]
    cb["whp1"] = whp[128:256]
    cb["ones128"] = np.ones((128, 1), np.float32)
    cb["neg1b"] = np.full((128, 1), -1.0, np.float32)

    cf = {}
    cf["neg1"] = np.full((128, 1), -1.0, np.float32)
    bh6 = (b_head - W_head.astype(np.float64).sum(0))         # [6]
    bh96 = np.tile(bh6, 16).reshape(96, 1)
    cf["bias_head"] = bh96.astype(np.float32)
    cf["bias_head_p1"] = (bh96 + 1.0).astype(np.float32)
    bhL = np.zeros((128, 1))
    for s in range(4):
        bhL[32 * s:32 * s + 6, 0] = bh6
    cf["bias_head_L"] = bhL.astype(np.float32)
    cf["bias_head_p1_L"] = (bhL + (bhL != 0)).astype(np.float32)

    # phase2 bias: b_ph2 - colsum(W_ph2)
    by4 = b_ph2 - W_ph2.astype(np.float64).sum(0)             # [4]
    by = np.tile(by4, 32).reshape(128, 1)
    cf["bias_y"] = by.astype(np.float32)
    cf["bias_y_p1"] = (by + 1.0).astype(np.float32)

    # pattern lhsTs over head tiles.  Head tile tau holds steps
    # t = 16*tau + tt at partitions 6*tt + r (r: 0,1 logits; 2..5 phase);
    # steps 60..63 (tau=3, tt>=12) live in the last PSUM bank instead and
    # are covered by the *_L patterns at partitions 32*s + r.
    pattD = np.zeros((96, 256), np.float32)
    pattW2 = np.zeros((96, 512), np.float32)
    for tau in range(4):
        for tt in range(16):
            t = 16 * tau + tt
            if t >= 60:
                continue
            pattD[6 * tt + 0, 64 * tau + t] = -1.0
            pattD[6 * tt + 1, 64 * tau + t] = 1.0
            h = tau // 2
            for i in range(4):
                for j in range(4):
                    q = 4 * (t - 32 * h) + j   # out partition within half h
                    pattW2[6 * tt + 2 + i, 128 * tau + q] = W_ph2[i, j]
    cb["pattD"] = pattD
    cb["pattW2"] = pattW2
    pattD_L = np.zeros((128, 64), np.float32)
    pattW2_L = np.zeros((128, 128), np.float32)
    for s in range(4):
        t = 60 + s
        pattD_L[32 * s + 0, t] = -1.0
        pattD_L[32 * s + 1, t] = 1.0
        for i in range(4):
            for j in range(4):
                q = 4 * (t - 32) + j
                pattW2_L[32 * s + 2 + i, q] = W_ph2[i, j]
    cb["pattD_L"] = pattD_L
    cb["pattW2_L"] = pattW2_L

    cbuf = np.zeros((128, _CB_COLS), np.float32)
    for k, (a, b) in _CB_OFF.items():
        arr = cb[k]
        cbuf[:arr.shape[0], a:b] = arr
    cfbuf = np.zeros((128, _CF_COLS), np.float32)
    for k, (a, b) in _CF_OFF.items():
        if k == "sx":
            continue
        arr = cf[k]
        cfbuf[:arr.shape[0], a:b] = arr
    return cbuf.astype(BF), cfbuf


_IN_SPECS = [
    # name, shape, dtype
    ("x_aug", (3, L * BL), BF16),
    ("cb", (128, _CB_COLS), BF16),
    ("cf", (128, _CF_COLS), F32),
]

# ---------------------------------------------------------------- device side

N_WARM = 5


def _build_kernel(ctx: ExitStack, tc: tile.TileContext, io: dict):
    nc = tc.nc
    sb = ctx.enter_context(tc.tile_pool(name="sb", bufs=1))
    st = ctx.enter_context(tc.tile_pool(name="st", bufs=6))
    tmp = ctx.enter_context(tc.tile_pool(name="tmp", bufs=44))
    spool = ctx.enter_context(tc.tile_pool(name="sg", bufs=4))
    ppool = ctx.enter_context(tc.tile_pool(name="pp", bufs=1))

    scratch = sb.tile([128, 513], BF16, tag="scratch", name="scratch")
    nc.gpsimd.memset(scratch[:, :], 0.0)
    # pre-load the one activation table holding both Exp and Ln
    # (natural_log_exp_and_others, id 6): the table-load pass then has no
    # mid-run switch to insert ahead of the post-phase Ln
    nc.scalar.add_instruction(mybir.InstLoadActFuncSet(
        name=nc.get_next_instruction_name(), ins=[], outs=[],
        act_func_set_id=6))

    cb = sb.tile([128, _CB_COLS], BF16, tag="cb", name="cb_sb")
    cf = sb.tile([128, _CF_COLS], F32, tag="cf", name="cf_sb")
    x_aug = sb.tile([3, L * BL], BF16, tag="x_aug", name="x_aug_sb")
    # scan-critical first so the recurrence starts ASAP: packed weights,
    # the tiny bias column block (every scan exp reads neg1), early x rows
    nsx = _CF_OFF["sx"][0]
    nc.sync.dma_start(cb[:, 0:_CB_CRIT], io["cb"][:, 0:_CB_CRIT])
    nc.sync.dma_start(cf[:, 0:nsx], io["cf"][:, 0:nsx])
    nc.sync.dma_start(x_aug[0:3, 0:L * BL // 2],
                      io["x_aug"][0:3, 0:L * BL // 2])
    nc.sync.dma_start(cb[:, _CB_CRIT:_CB_COLS], io["cb"][:, _CB_CRIT:_CB_COLS])
    nc.sync.dma_start(cf[:, nsx:_CF_COLS], io["cf"][:, nsx:_CF_COLS])
    nc.sync.dma_start(x_aug[0:3, L * BL // 2:L * BL],
                      io["x_aug"][0:3, L * BL // 2:L * BL])

    def cbs(name, rows=128):
        a, b = _CB_OFF[name]
        return cb[0:rows, a:b]

    def cfs(name, rows=128):
        a, b = _CF_OFF[name]
        return cf[0:rows, a:b]

    wc = [cbs("wc0"), cbs("wc1")]
    db = cbs("db", rows=3)
    nh0s = cbs("nh0")
    whp = [cbs("whp0"), cbs("whp1")]
    wh = [cbs("wh0"), cbs("wh1")]

    # headsP SBUF store: [96, 2048] bf16, section tau (cols 512*tau+) holds
    # steps 16*tau + tt at partitions 6*tt..6*tt+5 (only steps < 60)
    headsP = sb.tile([96, 2048], BF16, tag="headsP", name="headsP")

    nh_m1 = [nh0s] * NQ   # state t-1 (feeds carry matmuls)
    nh_m2 = [nh0s] * NQ   # state t-2 (feeds head matmuls)

    def emit_head(t_h, q, hps, kk):
        # head matmul k-chunk kk for quarter q of step t_h
        s = t_h % 4
        if s == 0:
            # zero-padded weights write the full bank partition-wise:
            # initializes junk partitions so the eviction reads defined data
            dst = hps[:, 128 * q:128 * q + 128]
            w = whp[kk]
        else:
            dst = hps[32 * s:32 * s + 6, 128 * q:128 * q + 128]
            w = wh[kk]
        nc.tensor.matmul(dst, w, nh_m2[q][:, 128 * kk:128 * kk + 128],
                         start=(kk == 0), stop=(kk == 1),
                         tile_position=(0, 32 * s))

    def flush_heads(t_last, hps):
        # steps t_last-3 .. t_last live at offsets 32*s.  DMA cannot read
        # PSUM, so evict the bank bf16 (split across DVE and ACT to halve
        # the intrusion into the scan tail cadence), then shuffle
        # partitions with SBUF->SBUF DMAs.
        stg = spool.tile([128, 512], BF16, tag="hstage",
                         name=f"hstage{t_last}")
        nc.vector.tensor_copy(stg[:, 0:256], hps[:, 0:256])
        nc.scalar.activation(stg[:, 256:512], hps[:, 256:512], AF.Copy)
        tau = (t_last - 3) // 16
        for s in range(4):
            tt = (t_last - 3 + s) % 16
            nc.sync.dma_start(
                headsP[6 * tt:6 * tt + 6, 512 * tau:512 * tau + 512],
                stg[32 * s:32 * s + 6, :])

    dpool = ctx.enter_context(tc.tile_pool(name="dp", bufs=1, space="PSUM"))
    D = dpool.tile([64, 512], F32, tag="D")
    nhd = []   # elu'd head tiles per tau, for the post phase
    he_tiles = {}

    hpool = ctx.enter_context(tc.tile_pool(name="hp", bufs=3, space="PSUM"))
    if True:
        hps = hpool.tile([128, 512], F32, tag="hps")

        with tc.tile_pool(name="z", bufs=1, space="PSUM") as zpool:
            # PE p-state warmup on junk data while the input DMAs land;
            # targets the D bank (its first real group re-initializes it)
            for i in range(N_WARM):
                nc.tensor.matmul(D[0:1, 0:512], scratch[:, 512:513],
                                 scratch[:, 0:512], start=True, stop=True)

            for t in range(1, L):
                for q in range(NQ):
                    zt = zpool.tile([128, 256], F32, tag=f"z{q}",
                                    name=f"z{q}_{t}")
                    zm = [zt[:, 0:128], zt[:, 128:256]]
                    xr = x_aug[0:3, 512 * (t - 1) + 128 * q:
                               512 * (t - 1) + 128 * q + 128]
                    nhp = nh_m1[q]
                    # input+bias matmuls and (t-2)-head matmuls carry no
                    # nh(t-1) dep, so they pad the front of the PE block
                    # while the previous stt drains; the four wc matmuls
                    # sit last, maximizing pipeline slack for the elu tail
                    nc.tensor.matmul(zm[0], db[:, 0:128], xr,
                                     start=True, stop=False)
                    if t >= 2:
                        emit_head(t - 2, q, hps, 0)
                        emit_head(t - 2, q, hps, 1)
                    nc.tensor.matmul(zm[0], wc[0][:, 0:128], nhp[:, 0:128],
                                     start=False, stop=False)
                    nc.tensor.matmul(zm[0], wc[1][:, 0:128], nhp[:, 128:256],
                                     start=False, stop=True)
                    nc.tensor.matmul(zm[1], db[:, 128:256], xr,
                                     start=True, stop=False)
                    nc.tensor.matmul(zm[1], wc[0][:, 128:256], nhp[:, 0:128],
                                     start=False, stop=False)
                    nc.tensor.matmul(zm[1], wc[1][:, 128:256],
                                     nhp[:, 128:256], start=False, stop=True)

                    e = tmp.tile([128, 256], BF16, tag=f"e{q}",
                                 name=f"e{q}_{t}")
                    nc.scalar.activation(e[:, :], zt[:, :], AF.Exp,
                                         bias=cbs("neg1b")[:, 0:1])
                    nh = st.tile([128, 256], BF16, tag=f"nh{q}",
                                 name=f"nh{q}_{t}")
                    # fused elu tail: nh = min(max(z+1, 1), exp(z))
                    nc.vector.scalar_tensor_tensor(nh[:, :], zt[:, :], 1.0,
                                                   e[:, :], OP.max, OP.min)
                    if t == L - 1:
                        # pull step-62 heads into the last block: the drain's
                        # hps completion then gates only on the 8 step-63
                        # head matmuls
                        nh_m2[q] = nh_m1[q]
                        emit_head(62, q, hps, 0)
                        emit_head(62, q, hps, 1)
                    nh_m2[q] = nh_m1[q]
                    nh_m1[q] = nh

                if t % 4 == 1 and t >= 5:
                    # heads t-5..t-2 are complete in this bank now
                    flush_heads(t - 2, hps)
                    hps = hpool.tile([128, 512], F32, tag="hps",
                                     name=f"hps_{t}")

                if t in (23, 39, 55, 24, 40, 56):
                    # headsP section tau is fully flushed two steps ago:
                    # pre-compute its elu'd head tile + D-matmul in scan
                    # slack, exp split over two steps to halve the ACT
                    # intrusion into the saturated scan cadence
                    tau = (t - 23) // 16
                    half = (t - 23) % 16
                    nhd_t = _post_tau(nc, ppool, headsP, cfs, cbs, D,
                                      tau, half, he_tiles)
                    if half == 1:
                        nhd.append(nhd_t)
                if t == L - 1:
                    # tau=3 section (steps 48..59) landed after the t=61
                    # flush; its elu chain overlaps the last block + drain
                    he_3 = ppool.tile([96, 512], BF16, tag="he3", name="he3")
                    nc.scalar.activation(he_3[0:72, :],
                                         headsP[0:72, 1536:2048], AF.Exp,
                                         bias=cfs("bias_head", rows=72)[:, 0:1])
                    hu_3 = ppool.tile([96, 512], BF16, tag="hu3", name="hu3")
                    nc.vector.tensor_scalar(
                        hu_3[0:72, :], headsP[0:72, 1536:2048],
                        cfs("bias_head_p1", rows=72)[:, 0:1], 1.0,
                        OP.add, OP.max)

        # ------------------------------------------------------- scan drain
        # heads 63 gate the L-path; the y0/D matmuls (deps ready since
        # mid-scan) are interleaved between quarters to fill the PE idle
        # gaps while each quarter's state drains
        pps = ctx.enter_context(tc.tile_pool(name="pps", bufs=1,
                                             space="PSUM"))
        y = [pps.tile([128, 512], F32, tag=f"y{h}", name=f"y{h}")
             for h in range(2)]
        pw2 = cbs("pattW2", rows=96)
        fillers = [
            lambda tau=tau: nc.tensor.matmul(
                y[0], pw2[:, 128 * tau:128 * tau + 128], nhd[tau][:, :],
                start=(tau == 0), stop=(tau == 1))
            for tau in range(2)
        ] + [
            lambda tau=tau: nc.tensor.matmul(
                D, cbs("pattD", rows=96)[:, 64 * tau:64 * tau + 64],
                nhd[tau][:, :], start=(tau == 0), stop=False)
            for tau in range(3)
        ]
        nh_m2 = nh_m1
        for q in range(NQ):
            for kk in range(2):
                emit_head(63, q, hps, kk)
            for f in fillers[2 * q:2 * q + 2]:
                f()

        # last bank (steps 60..63) post-processed directly from PSUM
        he_L = ppool.tile([128, 512], BF16, tag="he_L")
        nc.scalar.activation(he_L[:, :], hps[:, :], AF.Exp,
                             bias=cfs("bias_head_L")[:, 0:1])
        hu_L = ppool.tile([128, 512], BF16, tag="hu_L")
        nc.vector.tensor_scalar(hu_L[:, :], hps[:, :],
                                cfs("bias_head_p1_L")[:, 0:1], 1.0,
                                OP.add, OP.max)
        nhd_3 = ppool.tile([96, 512], BF16, tag="nhd3", name="nhd3")
        nc.vector.tensor_tensor(nhd_3[0:72, :], he_3[0:72, :],
                                hu_3[0:72, :], OP.min)
        nhd.append(nhd_3)
        nhd_L = ppool.tile([128, 512], BF16, tag="nhd_L")
        nc.vector.tensor_tensor(nhd_L[:, :], he_L[:, :], hu_L[:, :], OP.min)

        ye0 = ppool.tile([128, 512], BF16, tag="ye0")
        nc.scalar.activation(ye0[:, :], y[0][:, :], AF.Exp,
                             bias=cfs("bias_y")[:, 0:1])

        nc.tensor.matmul(D, cbs("pattD", rows=72)[:, 192:256],
                         nhd_3[0:72, :], start=False, stop=False)
        nc.tensor.matmul(D, cbs("pattD_L"), nhd_L[:, :],
                         start=False, stop=True)

    # ------------------------------------------------- phase + logp tails
    nc.tensor.matmul(y[1], pw2[:, 256:384], nhd[2][:, :],
                     start=True, stop=False)
    nc.tensor.matmul(y[1], pw2[0:72, 384:512], nhd[3][0:72, :],
                     start=False, stop=False)
    nc.tensor.matmul(y[1], cbs("pattW2_L"), nhd_L[:, :],
                     start=False, stop=True)

    # logp(t,b) = -softplus((1-2x)*D); summed over t by a matmul
    u = ppool.tile([64, 512], BF16, tag="u")
    nc.vector.scalar_tensor_tensor(u[:, :], D[:, :], 1.0,
                                   cfs("sx", rows=64)[:, :],
                                   OP.mult, OP.mult)
    ye1 = ppool.tile([128, 512], BF16, tag="ye1")
    nc.scalar.activation(ye1[:, :], y[1][:, :], AF.Exp,
                         bias=cfs("bias_y")[:, 0:1])
    # softplus(u) = ln(exp(u) + 1): Exp and Ln share one activation table
    # (AF.Softplus has no loadable table entry), and the +1 is Ln's bias
    eu = ppool.tile([64, 512], BF16, tag="eu")
    nc.scalar.activation(eu[:, :], u[:, :], AF.Exp)
    sp = ppool.tile([64, 512], BF16, tag="sp")
    nc.scalar.activation(sp[:, :], eu[:, :], AF.Ln, bias=1.0)

    sum_i = pps.tile([1, 512], F32, tag="sum_i")
    yu0 = ppool.tile([128, 512], BF16, tag="yu0")
    nc.vector.tensor_scalar(yu0[:, :], y[0][:, :],
                            cfs("bias_y_p1")[:, 0:1], 1.0, OP.add, OP.max)
    nh20 = ppool.tile([128, 512], BF16, tag="nh20")
    nc.vector.tensor_tensor(nh20[:, :], ye0[:, :], yu0[:, :], OP.min)
    nc.tensor.matmul(sum_i, cbs("ones128"), nh20[:, :],
                     start=True, stop=False)
    yu1 = ppool.tile([128, 512], BF16, tag="yu1")
    nc.vector.tensor_scalar(yu1[:, :], y[1][:, :],
                            cfs("bias_y_p1")[:, 0:1], 1.0, OP.add, OP.max)
    nh21 = ppool.tile([128, 512], BF16, tag="nh21")
    nc.vector.tensor_tensor(nh21[:, :], ye1[:, :], yu1[:, :], OP.min)
    nc.tensor.matmul(sum_i, cbs("ones128"), nh21[:, :],
                     start=False, stop=True)

    # softplus partition-sum straight to the staging tile on the idle
    # GPSIMD engine (negated on the host); the phase sum copies via ACT
    stage = ppool.tile([1, 1024], F32, tag="stage")
    nc.gpsimd.tensor_reduce(stage[0:1, 0:512], sp[:, :],
                            mybir.AxisListType.C, OP.add)
    nc.scalar.activation(stage[0:1, 512:1024], sum_i[:, :], AF.Copy)
    nc.sync.dma_start(io["out"][0:1, :], stage[0:1, :])


def _post_tau(nc, ppool, headsP, cfs, cbs, D, tau, half, he_tiles):
    # elu'd (shifted) head tile for section tau + its D matmul; the exp is
    # emitted in column halves on consecutive steps (half=0, then half=1
    # finishes the tile and runs the combine + matmul)
    nr = 96
    c0, c1 = 512 * tau + 256 * half, 512 * tau + 256 * half + 256
    hsec = headsP[0:nr, c0:c1]
    if half == 0:
        he = ppool.tile([96, 512], BF16, tag=f"he{tau}", name=f"he{tau}")
        he_tiles[tau] = he
    else:
        he = he_tiles[tau]
    nc.scalar.activation(he[0:nr, 256 * half:256 * half + 256], hsec, AF.Exp,
                         bias=cfs("bias_head", rows=nr)[:, 0:1])
    if half == 0:
        return None
    hu = ppool.tile([96, 512], BF16, tag=f"hu{tau}", name=f"hu{tau}")
    nc.gpsimd.tensor_scalar(hu[0:nr, :],
                            headsP[0:nr, 512 * tau:512 * tau + 512],
                            cfs("bias_head_p1", rows=nr)[:, 0:1], 1.0,
                            OP.add, OP.max)
    nhd = ppool.tile([96, 512], BF16, tag=f"nhd{tau}", name=f"nhd{tau}")
    nc.vector.tensor_tensor(nhd[0:nr, :], he[0:nr, :], hu[0:nr, :], OP.min)
    return nhd


def build_program():
    nc = bacc.Bacc("TRN2", target_bir_lowering=False, debug=False,
                   num_devices=NCORES)
    io = {}
    for name, shape, dt in _IN_SPECS:
        io[name] = nc.dram_tensor(name, list(shape), dt,
                                  kind="ExternalInput").ap()
    io["out"] = nc.dram_tensor("out", [1, 2 * BL], F32,
                               kind="ExternalOutput").ap()
    with tile.TileContext(nc) as tc:
        with ExitStack() as ctx:
            _build_kernel(ctx, tc, io)
    nc.compile()
    return nc


def make_in_maps(x, W_in, W_carry, b_carry, W_prob, b_prob, W_ph1, b_ph1,
                 W_ph2, b_ph2):
    cbuf, cfbuf = _host_constants(W_in, W_carry, b_carry, W_prob, b_prob,
                                  W_ph1, b_ph1, W_ph2, b_ph2)
    in_maps = []
    a, b = _CF_OFF["sx"]
    for c in range(NCORES):
        xs = np.ascontiguousarray(x[c * BL:(c + 1) * BL].T)  # [64, 512] i32
        xa = np.ones((3, L * BL), np.float32)
        xa[0] = xs.astype(np.float32).reshape(-1)
        cfc = cfbuf.copy()
        cfc[0:64, a:b] = 1.0 - 2.0 * xs.astype(np.float32)
        in_maps.append({"x_aug": xa.astype(BF), "cb": cbuf, "cf": cfc})
    return in_maps


_PROGRAM = None


def kernel(x, W_in, W_carry, b_carry, W_prob, b_prob, W_ph1, b_ph1, W_ph2,
           b_ph2):
    global _PROGRAM
    x = np.asarray(x)
    in_maps = make_in_maps(x, np.asarray(W_in), np.asarray(W_carry),
                           np.asarray(b_carry), np.asarray(W_prob),
                           np.asarray(b_prob), np.asarray(W_ph1),
                           np.asarray(b_ph1), np.asarray(W_ph2),
                           np.asarray(b_ph2))
    if _PROGRAM is None:
        _PROGRAM = build_program()
    res = run_bass_kernel_spmd(_PROGRAM, in_maps, core_ids=list(range(NCORES)))
    outs = [np.asarray(res.results[c]["out"])[0] for c in range(NCORES)]
    real = -0.5 * np.concatenate([o[0:BL] for o in outs])
    imag = (np.concatenate([o[BL:2 * BL] for o in outs]) - 256.0) / 256.0
    return (real + 1j * imag).astype(np.complex64)


# revision 67
# speedup vs baseline: 1.0025x; 1.0025x over previous
"""Trainium2 Bass kernel for nn_CpxRNN: 64-step RNN over B=4096 samples,
data-parallel across 8 NeuronCores (512 samples/core).

Math (per core, b = sample, columns of every on-chip tile):
  state kept transposed+shifted: nh = (elu(z)+1)^T, four independent
  quarter-chains of 128 samples (tiles [128, 256]; hidden chunk m in cols
  [128m, 128m+128), hidden unit i = 128*m + p).  The exp+stt elu tail of
  quarter q overlaps the other quarters' matmuls, keeping PE
  throughput-bound (~32 x 53ns matmuls/step) instead of latency-bound.
  elu(z)+1 == min(exp(z), max(z+1, 1))  -- 1 ACT + 1 DVE op, bias folded
  into the K=3 augmented input matmul (all "-1" shift corrections folded
  into host-precomputed biases b~ = b - colsum(W)).
  One-hot input term reduced to rank-1: prevoh @ W_in = W_in[0] + x*delta.
  Heads (logits 2 + phase 4 rows) for step t-2 are emitted during step t
  (2 matmuls per quarter at 32-aligned partition offsets, 4 steps per PSUM
  bank, bank evicted bf16 via split DVE/ACT copy + SBUF DMA shuffle).
  The final bank (steps 60..63) is post-processed directly from PSUM.
  Post phase: logp(t,b) = -softplus((1-2x)*D) with D = L1-L0 from a
  pattern matmul over elu'd head tiles (binary log-softmax identity
  removes the S / L0 / log-softmax tensors); softplus = Ln(Exp(u)+1) on
  one pre-loaded activation table (no mid-run table switch); final sums
  via one PE matmul + a GPSIMD partition-reduce straight into the DMA
  staging tile.  tau 0..2 head post-processing is injected into scan
  slack; the final PSUM bank (steps 60..63) is consumed in place.
  All constants ride in two packed dram tensors (5 input DMAs total);
  dummy warmup matmuls ramp the PE p-state during the DMA wait.
"""

import sys

sys.path.insert(0, "/opt/trn_rl_repo")

from contextlib import ExitStack

import ml_dtypes
import numpy as np

import concourse.bass as bass
import concourse.tile as tile
from concourse import bacc, mybir
from concourse.bass_utils import run_bass_kernel_spmd

L = 64
H = 256
B = 4096
NCORES = 8
BL = B // NCORES  # 512
NQ = 4            # quarter-chains per core, 128 samples each
F32 = mybir.dt.float32
BF16 = mybir.dt.bfloat16
AF = mybir.ActivationFunctionType
OP = mybir.AluOpType
BF = ml_dtypes.bfloat16

# packed-constant column layouts (bf16 tensor "cb", f32 tensor "cf");
# (name, ncols).  Rows beyond each entry's natural height are zero.
_CB_LAYOUT = [
    ("wc0", 256), ("wc1", 256), ("db", 256), ("nh0", 256),      # scan-crit
    ("neg1b", 1), ("whp0", 128), ("whp1", 128), ("wh0", 6), ("wh1", 6),
    ("ones128", 1),
    ("pattD", 256), ("pattW2", 512), ("pattD_L", 64), ("pattW2_L", 128),
]
_CB_CRIT = 1281   # cols holding scan-critical constants (first DMA)
_CF_LAYOUT = [
    ("neg1", 1), ("bias_head", 1), ("bias_head_p1", 1),
    ("bias_y", 1), ("bias_y_p1", 1),
    ("bias_head_L", 1), ("bias_head_p1_L", 1), ("sx", 512),
]


def _offsets(layout):
    offs, c = {}, 0
    for k, n in layout:
        offs[k] = (c, c + n)
        c += n
    return offs, c


_CB_OFF, _CB_COLS = _offsets(_CB_LAYOUT)
_CF_OFF, _CF_COLS = _offsets(_CF_LAYOUT)

# ---------------------------------------------------------------- host side


def _host_constants(W_in, W_carry, b_carry, W_prob, b_prob, W_ph1, b_ph1,
                    W_ph2, b_ph2):
    W_in = W_in.astype(np.float64)
    W_carry = W_carry.astype(np.float64)
    # state bias: b_carry - colsum(W_carry) + W_in[0]
    bias_state_vec = b_carry - W_carry.sum(0) + W_in[0]

    cb = {}
    # initial shifted state (t=0 uses zero prev input, zero h), quarter layout
    nh0_vec = np.where(b_carry > 0, b_carry, np.expm1(b_carry)) + 1.0  # [256]
    nh0 = np.empty((128, 256), np.float32)
    for m in range(2):
        nh0[:, 128 * m:128 * m + 128] = nh0_vec[128 * m:128 * m + 128][:, None]
    cb["nh0"] = nh0

    wcf = W_carry.astype(np.float32)
    cb["wc0"] = wcf[0:128]
    cb["wc1"] = wcf[128:256]
    # K=3 augmented input matmul: lhsT rows = [delta; bias_hi; bias_lo]
    # (bias split so the bf16 lhsT carries it at ~f32 accuracy)
    bp1 = bias_state_vec + 1.0   # PSUM carries z+1 for the fused elu tail
    b_hi = bp1.astype(np.float32).astype(BF).astype(np.float64)
    b_lo = bp1 - b_hi
    cb["db"] = np.stack([W_in[1] - W_in[0], b_hi, b_lo]).astype(np.float32)

    W_head = np.concatenate([W_prob, W_ph1], axis=1)          # [256, 6]
    b_head = np.concatenate([b_prob, b_ph1])                  # [6]
    cb["wh0"] = W_head[0:128].astype(np.float32)
    cb["wh1"] = W_head[128:256].astype(np.float32)
    whp = np.zeros((256, 128), np.float32)
    whp[:, :6] = W_head
    cb["whp0"] = whp[0:128]
    cb["whp1"] = whp[128:256]
    cb["ones128"] = np.ones((128, 1), np.float32)
    cb["neg1b"] = np.full((128, 1), -1.0, np.float32)

    cf = {}
    cf["neg1"] = np.full((128, 1), -1.0, np.float32)
    bh6 = (b_head - W_head.astype(np.float64).sum(0))         # [6]
    bh96 = np.tile(bh6, 16).reshape(96, 1)
    cf["bias_head"] = bh96.astype(np.float32)
    cf["bias_head_p1"] = (bh96 + 1.0).astype(np.float32)
    bhL = np.zeros((128, 1))
    for s in range(4):
        bhL[32 * s:32 * s + 6, 0] = bh6
    cf["bias_head_L"] = bhL.astype(np.float32)
    cf["bias_head_p1_L"] = (bhL + (bhL != 0)).astype(np.float32)

    # phase2 bias: b_ph2 - colsum(W_ph2)
    by4 = b_ph2 - W_ph2.astype(np.float64).sum(0)             # [4]
    by = np.tile(by4, 32).reshape(128, 1)
    cf["bias_y"] = by.astype(np.float32)
    cf["bias_y_p1"] = (by + 1.0).astype(np.float32)

    # pattern lhsTs over head tiles.  Head tile tau holds steps
    # t = 16*tau + tt at partitions 6*tt + r (r: 0,1 logits; 2..5 phase);
    # steps 60..63 (tau=3, tt>=12) live in the last PSUM bank instead and
    # are covered by the *_L patterns at partitions 32*s + r.
    pattD = np.zeros((96, 256), np.float32)
    pattW2 = np.zeros((96, 512), np.float32)
    for tau in range(4):
        for tt in range(16):
            t = 16 * tau + tt
            if t >= 60:
                continue
            pattD[6 * tt + 0, 64 * tau + t] = -1.0
            pattD[6 * tt + 1, 64 * tau + t] = 1.0
            h = tau // 2
            for i in range(4):
                for j in range(4):
                    q = 4 * (t - 32 * h) + j   # out partition within half h
                    pattW2[6 * tt + 2 + i, 128 * tau + q] = W_ph2[i, j]
    cb["pattD"] = pattD
    cb["pattW2"] = pattW2
    pattD_L = np.zeros((128, 64), np.float32)
    pattW2_L = np.zeros((128, 128), np.float32)
    for s in range(4):
        t = 60 + s
        pattD_L[32 * s + 0, t] = -1.0
        pattD_L[32 * s + 1, t] = 1.0
        for i in range(4):
            for j in range(4):
                q = 4 * (t - 32) + j
                pattW2_L[32 * s + 2 + i, q] = W_ph2[i, j]
    cb["pattD_L"] = pattD_L
    cb["pattW2_L"] = pattW2_L

    cbuf = np.zeros((128, _CB_COLS), np.float32)
    for k, (a, b) in _CB_OFF.items():
        arr = cb[k]
        cbuf[:arr.shape[0], a:b] = arr
    cfbuf = np.zeros((128, _CF_COLS), np.float32)
    for k, (a, b) in _CF_OFF.items():
        if k == "sx":
            continue
        arr = cf[k]
        cfbuf[:arr.shape[0], a:b] = arr
    return cbuf.astype(BF), cfbuf


_IN_SPECS = [
    # name, shape, dtype
    ("x_aug", (3, L * BL), BF16),
    ("cb", (128, _CB_COLS), BF16),
    ("cf", (128, _CF_COLS), F32),
]

# ---------------------------------------------------------------- device side

N_WARM = 5


def _build_kernel(ctx: ExitStack, tc: tile.TileContext, io: dict):
    nc = tc.nc
    sb = ctx.enter_context(tc.tile_pool(name="sb", bufs=1))
    st = ctx.enter_context(tc.tile_pool(name="st", bufs=6))
    tmp = ctx.enter_context(tc.tile_pool(name="tmp", bufs=44))
    spool = ctx.enter_context(tc.tile_pool(name="sg", bufs=4))
    ppool = ctx.enter_context(tc.tile_pool(name="pp", bufs=1))

    scratch = sb.tile([128, 513], BF16, tag="scratch", name="scratch")
    nc.gpsimd.memset(scratch[:, :], 0.0)
    # pre-load the one activation table holding both Exp and Ln
    # (natural_log_exp_and_others, id 6): the table-load pass then has no
    # mid-run switch to insert ahead of the post-phase Ln
    nc.scalar.add_instruction(mybir.InstLoadActFuncSet(
        name=nc.get_next_instruction_name(), ins=[], outs=[],
        act_func_set_id=6))

    cb = sb.tile([128, _CB_COLS], BF16, tag="cb", name="cb_sb")
    cf = sb.tile([128, _CF_COLS], F32, tag="cf", name="cf_sb")
    x_aug = sb.tile([3, L * BL], BF16, tag="x_aug", name="x_aug_sb")
    # scan-critical first so the recurrence starts ASAP: packed weights,
    # the tiny bias column block (every scan exp reads neg1), early x rows
    nsx = _CF_OFF["sx"][0]
    nc.sync.dma_start(cb[:, 0:_CB_CRIT], io["cb"][:, 0:_CB_CRIT])
    nc.sync.dma_start(cf[:, 0:nsx], io["cf"][:, 0:nsx])
    nc.sync.dma_start(x_aug[0:3, 0:L * BL // 2],
                      io["x_aug"][0:3, 0:L * BL // 2])
    nc.sync.dma_start(cb[:, _CB_CRIT:_CB_COLS], io["cb"][:, _CB_CRIT:_CB_COLS])
    nc.sync.dma_start(cf[:, nsx:_CF_COLS], io["cf"][:, nsx:_CF_COLS])
    nc.sync.dma_start(x_aug[0:3, L * BL // 2:L * BL],
                      io["x_aug"][0:3, L * BL // 2:L * BL])

    def cbs(name, rows=128):
        a, b = _CB_OFF[name]
        return cb[0:rows, a:b]

    def cfs(name, rows=128):
        a, b = _CF_OFF[name]
        return cf[0:rows, a:b]

    wc = [cbs("wc0"), cbs("wc1")]
    db = cbs("db", rows=3)
    nh0s = cbs("nh0")
    whp = [cbs("whp0"), cbs("whp1")]
    wh = [cbs("wh0"), cbs("wh1")]

    # headsP SBUF store: [96, 2048] bf16, section tau (cols 512*tau+) holds
    # steps 16*tau + tt at partitions 6*tt..6*tt+5 (only steps < 60)
    headsP = sb.tile([96, 2048], BF16, tag="headsP", name="headsP")

    nh_m1 = [nh0s] * NQ   # state t-1 (feeds carry matmuls)
    nh_m2 = [nh0s] * NQ   # state t-2 (feeds head matmuls)

    def emit_head(t_h, q, hps, kk):
        # head matmul k-chunk kk for quarter q of step t_h
        s = t_h % 4
        if s == 0:
            # zero-padded weights write the full bank partition-wise:
            # initializes junk partitions so the eviction reads defined data
            dst = hps[:, 128 * q:128 * q + 128]
            w = whp[kk]
        else:
            dst = hps[32 * s:32 * s + 6, 128 * q:128 * q + 128]
            w = wh[kk]
        nc.tensor.matmul(dst, w, nh_m2[q][:, 128 * kk:128 * kk + 128],
                         start=(kk == 0), stop=(kk == 1),
                         tile_position=(0, 32 * s))

    def flush_heads(t_last, hps):
        # steps t_last-3 .. t_last live at offsets 32*s.  DMA cannot read
        # PSUM, so evict the bank bf16 (split across DVE and ACT to halve
        # the intrusion into the scan tail cadence), then shuffle
        # partitions with SBUF->SBUF DMAs.
        stg = spool.tile([128, 512], BF16, tag="hstage",
                         name=f"hstage{t_last}")
        nc.vector.tensor_copy(stg[:, 0:256], hps[:, 0:256])
        nc.scalar.activation(stg[:, 256:512], hps[:, 256:512], AF.Copy)
        tau = (t_last - 3) // 16
        for s in range(4):
            tt = (t_last - 3 + s) % 16
            nc.sync.dma_start(
                headsP[6 * tt:6 * tt + 6, 512 * tau:512 * tau + 512],
                stg[32 * s:32 * s + 6, :])

    dpool = ctx.enter_context(tc.tile_pool(name="dp", bufs=1, space="PSUM"))
    D = dpool.tile([64, 512], F32, tag="D")
    nhd = []   # elu'd head tiles per tau, for the post phase
    he_tiles = {}

    hpool = ctx.enter_context(tc.tile_pool(name="hp", bufs=3, space="PSUM"))
    if True:
        hps = hpool.tile([128, 512], F32, tag="hps")

        with tc.tile_pool(name="z", bufs=1, space="PSUM") as zpool:
            # PE p-state warmup on junk data while the input DMAs land;
            # targets the D bank (its first real group re-initializes it)
            for i in range(N_WARM):
                nc.tensor.matmul(D[0:1, 0:512], scratch[:, 512:513],
                                 scratch[:, 0:512], start=True, stop=True)

            for t in range(1, L):
                for q in range(NQ):
                    zt = zpool.tile([128, 256], F32, tag=f"z{q}",
                                    name=f"z{q}_{t}")
                    zm = [zt[:, 0:128], zt[:, 128:256]]
                    xr = x_aug[0:3, 512 * (t - 1) + 128 * q:
                               512 * (t - 1) + 128 * q + 128]
                    nhp = nh_m1[q]
                    # input+bias matmuls and (t-2)-head matmuls carry no
                    # nh(t-1) dep, so they pad the front of the PE block
                    # while the previous stt drains; the four wc matmuls
                    # sit last, maximizing pipeline slack for the elu tail
                    nc.tensor.matmul(zm[0], db[:, 0:128], xr,
                                     start=True, stop=False)
                    if t >= 2:
                        emit_head(t - 2, q, hps, 0)
                        emit_head(t - 2, q, hps, 1)
                    nc.tensor.matmul(zm[0], wc[0][:, 0:128], nhp[:, 0:128],
                                     start=False, stop=False)
                    nc.tensor.matmul(zm[0], wc[1][:, 0:128], nhp[:, 128:256],
                                     start=False, stop=True)
                    nc.tensor.matmul(zm[1], db[:, 128:256], xr,
                                     start=True, stop=False)
                    nc.tensor.matmul(zm[1], wc[0][:, 128:256], nhp[:, 0:128],
                                     start=False, stop=False)
                    nc.tensor.matmul(zm[1], wc[1][:, 128:256],
                                     nhp[:, 128:256], start=False, stop=True)

                    e = tmp.tile([128, 256], BF16, tag=f"e{q}",
                                 name=f"e{q}_{t}")
                    nc.scalar.activation(e[:, :], zt[:, :], AF.Exp,
                                         bias=cbs("neg1b")[:, 0:1])
                    nh = st.tile([128, 256], BF16, tag=f"nh{q}",
                                 name=f"nh{q}_{t}")
                    # fused elu tail: nh = min(max(z+1, 1), exp(z))
                    nc.vector.scalar_tensor_tensor(nh[:, :], zt[:, :], 1.0,
                                                   e[:, :], OP.max, OP.min)
                    if t == L - 1:
                        # pull step-62 heads into the last block: the drain's
                        # hps completion then gates only on the 8 step-63
                        # head matmuls
                        nh_m2[q] = nh_m1[q]
                        emit_head(62, q, hps, 0)
                        emit_head(62, q, hps, 1)
                    nh_m2[q] = nh_m1[q]
                    nh_m1[q] = nh

                if t % 4 == 1 and t >= 5:
                    # heads t-5..t-2 are complete in this bank now
                    flush_heads(t - 2, hps)
                    hps = hpool.tile([128, 512], F32, tag="hps",
                                     name=f"hps_{t}")

                if t in (23, 39, 55, 24, 40, 56):
                    # headsP section tau is fully flushed two steps ago:
                    # pre-compute its elu'd head tile + D-matmul in scan
                    # slack, exp split over two steps to halve the ACT
                    # intrusion into the saturated scan cadence
                    tau = (t - 23) // 16
                    half = (t - 23) % 16
                    nhd_t = _post_tau(nc, ppool, headsP, cfs, cbs, D,
                                      tau, half, he_tiles)
                    if half == 1:
                        nhd.append(nhd_t)
                if t == L - 1:
                    # tau=3 section (steps 48..59) landed after the t=61
                    # flush; its elu chain overlaps the last block + drain
                    he_3 = ppool.tile([96, 512], BF16, tag="he3", name="he3")
                    nc.scalar.activation(he_3[0:72, :],
                                         headsP[0:72, 1536:2048], AF.Exp,
                                         bias=cfs("bias_head", rows=72)[:, 0:1])
                    hu_3 = ppool.tile([96, 512], BF16, tag="hu3", name="hu3")
                    nc.vector.tensor_scalar(
                        hu_3[0:72, :], headsP[0:72, 1536:2048],
                        cfs("bias_head_p1", rows=72)[:, 0:1], 1.0,
                        OP.add, OP.max)

        # ------------------------------------------------------- scan drain
        # heads 63 first: hps completion gates the whole L-path chain
        nh_m2 = nh_m1
        for q in range(NQ):
            for kk in range(2):
                emit_head(63, q, hps, kk)

        # phase-head half 0 and the tau 0..2 D-matmuls only need nhd
        # tiles ready since mid-scan: they fill the idle PE here instead
        # of intruding on the scan
        pps = ctx.enter_context(tc.tile_pool(name="pps", bufs=1,
                                             space="PSUM"))
        y = [pps.tile([128, 512], F32, tag=f"y{h}", name=f"y{h}")
             for h in range(2)]
        pw2 = cbs("pattW2", rows=96)
        for tau in range(2):
            nc.tensor.matmul(y[0], pw2[:, 128 * tau:128 * tau + 128],
                             nhd[tau][:, :], start=(tau == 0),
                             stop=(tau == 1))
        for tau in range(3):
            nc.tensor.matmul(D, cbs("pattD", rows=96)[:, 64 * tau:64 * tau + 64],
                             nhd[tau][:, :], start=(tau == 0), stop=False)

        # last bank (steps 60..63) post-processed directly from PSUM
        he_L = ppool.tile([128, 512], BF16, tag="he_L")
        nc.scalar.activation(he_L[:, :], hps[:, :], AF.Exp,
                             bias=cfs("bias_head_L")[:, 0:1])
        hu_L = ppool.tile([128, 512], BF16, tag="hu_L")
        nc.vector.tensor_scalar(hu_L[:, :], hps[:, :],
                                cfs("bias_head_p1_L")[:, 0:1], 1.0,
                                OP.add, OP.max)
        nhd_3 = ppool.tile([96, 512], BF16, tag="nhd3", name="nhd3")
        nc.vector.tensor_tensor(nhd_3[0:72, :], he_3[0:72, :],
                                hu_3[0:72, :], OP.min)
        nhd.append(nhd_3)
        nhd_L = ppool.tile([128, 512], BF16, tag="nhd_L")
        nc.vector.tensor_tensor(nhd_L[:, :], he_L[:, :], hu_L[:, :], OP.min)

        ye0 = ppool.tile([128, 512], BF16, tag="ye0")
        nc.scalar.activation(ye0[:, :], y[0][:, :], AF.Exp,
                             bias=cfs("bias_y")[:, 0:1])

        nc.tensor.matmul(D, cbs("pattD", rows=72)[:, 192:256],
                         nhd_3[0:72, :], start=False, stop=False)
        nc.tensor.matmul(D, cbs("pattD_L"), nhd_L[:, :],
                         start=False, stop=True)

    # ------------------------------------------------- phase + logp tails
    nc.tensor.matmul(y[1], pw2[:, 256:384], nhd[2][:, :],
                     start=True, stop=False)
    nc.tensor.matmul(y[1], pw2[0:72, 384:512], nhd[3][0:72, :],
                     start=False, stop=False)
    nc.tensor.matmul(y[1], cbs("pattW2_L"), nhd_L[:, :],
                     start=False, stop=True)

    # logp(t,b) = -softplus((1-2x)*D); summed over t by a matmul
    u = ppool.tile([64, 512], BF16, tag="u")
    nc.vector.scalar_tensor_tensor(u[:, :], D[:, :], 1.0,
                                   cfs("sx", rows=64)[:, :],
                                   OP.mult, OP.mult)
    ye1 = ppool.tile([128, 512], BF16, tag="ye1")
    nc.scalar.activation(ye1[:, :], y[1][:, :], AF.Exp,
                         bias=cfs("bias_y")[:, 0:1])
    # softplus(u) = ln(exp(u) + 1): Exp and Ln share one activation table
    # (AF.Softplus has no loadable table entry), and the +1 is Ln's bias
    eu = ppool.tile([64, 512], BF16, tag="eu")
    nc.scalar.activation(eu[:, :], u[:, :], AF.Exp)
    sp = ppool.tile([64, 512], BF16, tag="sp")
    nc.scalar.activation(sp[:, :], eu[:, :], AF.Ln, bias=1.0)

    sum_i = pps.tile([1, 512], F32, tag="sum_i")
    yu0 = ppool.tile([128, 512], BF16, tag="yu0")
    nc.vector.tensor_scalar(yu0[:, :], y[0][:, :],
                            cfs("bias_y_p1")[:, 0:1], 1.0, OP.add, OP.max)
    nh20 = ppool.tile([128, 512], BF16, tag="nh20")
    nc.vector.tensor_tensor(nh20[:, :], ye0[:, :], yu0[:, :], OP.min)
    nc.tensor.matmul(sum_i, cbs("ones128"), nh20[:, :],
                     start=True, stop=False)
    yu1 = ppool.tile([128, 512], BF16, tag="yu1")
    nc.vector.tensor_scalar(yu1[:, :], y[1][:, :],
                            cfs("bias_y_p1")[:, 0:1], 1.0, OP.add, OP.max)
    nh21 = ppool.tile([128, 512], BF16, tag="nh21")
    nc.vector.tensor_tensor(nh21[:, :], ye1[:, :], yu1[:, :], OP.min)
    nc.tensor.matmul(sum_i, cbs("ones128"), nh21[:, :],
                     start=False, stop=True)

    # softplus partition-sum straight to the staging tile on the idle
    # GPSIMD engine (negated on the host); the phase sum copies via ACT
    stage = ppool.tile([1, 1024], F32, tag="stage")
    nc.gpsimd.tensor_reduce(stage[0:1, 0:512], sp[:, :],
                            mybir.AxisListType.C, OP.add)
    nc.scalar.activation(stage[0:1, 512:1024], sum_i[:, :], AF.Copy)
    nc.sync.dma_start(io["out"][0:1, :], stage[0:1, :])


def _post_tau(nc, ppool, headsP, cfs, cbs, D, tau, half, he_tiles):
    # elu'd (shifted) head tile for section tau + its D matmul; the exp is
    # emitted in column halves on consecutive steps (half=0, then half=1
    # finishes the tile and runs the combine + matmul)
    nr = 96
    c0, c1 = 512 * tau + 256 * half, 512 * tau + 256 * half + 256
    hsec = headsP[0:nr, c0:c1]
    if half == 0:
        he = ppool.tile([96, 512], BF16, tag=f"he{tau}", name=f"he{tau}")
        he_tiles[tau] = he
    else:
        he = he_tiles[tau]
    nc.scalar.activation(he[0:nr, 256 * half:256 * half + 256], hsec, AF.Exp,
                         bias=cfs("bias_head", rows=nr)[:, 0:1])
    if half == 0:
        return None
    hu = ppool.tile([96, 512], BF16, tag=f"hu{tau}", name=f"hu{tau}")
    nc.gpsimd.tensor_scalar(hu[0:nr, :],
                            headsP[0:nr, 512 * tau:512 * tau + 512],
                            cfs("bias_head_p1", rows=nr)[:, 0:1], 1.0,
                            OP.add, OP.max)
    nhd = ppool.tile([96, 512], BF16, tag=f"nhd{tau}", name=f"nhd{tau}")
    nc.vector.tensor_tensor(nhd[0:nr, :], he[0:nr, :], hu[0:nr, :], OP.min)
    return nhd


def build_program():
    nc = bacc.Bacc("TRN2", target_bir_lowering=False, debug=False,
                   num_devices=NCORES)
    io = {}
    for name, shape, dt in _IN_SPECS:
        io[name] = nc.dram_tensor(name, list(shape), dt,
                                  kind="ExternalInput").ap()
    io["out"] = nc.dram_tensor("out", [1, 2 * BL], F32,
                               kind="ExternalOutput").ap()
    with tile.TileContext(nc) as tc:
        with ExitStack() as ctx:
            _build_kernel(ctx, tc, io)
    nc.compile()
    return nc


def make_in_maps(x, W_in, W_carry, b_carry, W_prob, b_prob, W_ph1, b_ph1,
                 W_ph2, b_ph2):
    cbuf, cfbuf = _host_constants(W_in, W_carry, b_carry, W_prob, b_prob,
                                  W_ph1, b_ph1, W_ph2, b_ph2)
    in_maps = []
    a, b = _CF_OFF["sx"]
    for c in range(NCORES):
        xs = np.ascontiguousarray(x[c * BL:(c + 1) * BL].T)  # [64, 512] i32
        xa = np.ones((3, L * BL), np.float32)
        xa[0] = xs.astype(np.float32).reshape(-1)
        cfc = cfbuf.copy()
        cfc[0:64, a:b] = 1.0 - 2.0 * xs.astype(np.float32)
        in_maps.append({"x_aug": xa.astype(BF), "cb": cbuf, "cf": cfc})
    return in_maps


_PROGRAM = None


def kernel(x, W_in, W_carry, b_carry, W_prob, b_prob, W_ph1, b_ph1, W_ph2,
           b_ph2):
    global _PROGRAM
    x = np.asarray(x)
    in_maps = make_in_maps(x, np.asarray(W_in), np.asarray(W_carry),
                           np.asarray(b_carry), np.asarray(W_prob),
                           np.asarray(b_prob), np.asarray(W_ph1),
                           np.asarray(b_ph1), np.asarray(W_ph2),
                           np.asarray(b_ph2))
    if _PROGRAM is None:
        _PROGRAM = build_program()
    res = run_bass_kernel_spmd(_PROGRAM, in_maps, core_ids=list(range(NCORES)))
    outs = [np.asarray(res.results[c]["out"])[0] for c in range(NCORES)]
    real = -0.5 * np.concatenate([o[0:BL] for o in outs])
    imag = (np.concatenate([o[BL:2 * BL] for o in outs]) - 256.0) / 256.0
    return (real + 1j * imag).astype(np.complex64)
